# revision 1
# baseline (speedup 1.0000x reference)
"""DTW layer (short kernel) Trainium2 Bass kernel.

Problem: x (B=8, C=8, L=4096) f32, kernels (F=32, K=10) f32.
For each (b, c, f, w): DTW cost between kernels[f] (len 10) and window
x[b, c, 5w : 5w+20], for w in [0, 815). Output (B, C*F, 815) f32.

Sharding: data-parallel over batch - core b computes batch b entirely
(C*F = 256 (c,f) combos = 2 partition chunks of 128).

Algorithm (per core): the DTW row recurrence
    row_i[j] = D[i,j] + min(row_i[j-1], row_{i-1}[j], row_{i-1}[j-1])
is computed for 128 (c,f) combos at once (partition dim) and a chunk of
windows laid out along the free dim as [w, 21] segments (1 separator +
20 cells).  Per row:
  - ACT computes local costs D[w, 1+j] = (x[5w+j] - k_i)^2 via
    activation(Square, bias=-k_i) with an overlapping strided input AP.
  - DVE computes m[t] = min(S_prev[t], S_prev[t-1]) (3-way-min helper)
    and then one tensor_tensor_scan per row:
    state = min(m[t], state) + D[t], with a BIG value in the separator
    column of D forcing a carry reset between windows.

Perf notes (hardware-measured in situ):
  - All DP tensors (S, D) are fp16: the shifted-min tensor_tensor runs
    ~1.5x faster (2x_1p packing) and scan speed is dtype-neutral.
    Max DP value ~70 << fp16 max; rel err ~1e-3 << 2e-2 gate.
  - The m helper writes INTO the scan's own output buffer S[i%2] (row
    i-2's values are dead by then) and the scan runs in place
    (data0 == out): one less distinct SBUF stream per scan (~6%).
  - Pair-interleaved units with same-op-type grouping (m,m,scan,scan)
    to hide cross-engine latency and reduce op-type switches.
  - m runs full-width (one contiguous 2D run incl. separators), which
    beats cells-only 3D windowed views; overwritten separator positions
    stay >= BIG, preserving the carry reset.

Raw bass (no Tile framework): engines are programmed directly with
standalone wait_ge instructions and per-engine semaphores.
"""

from contextlib import ExitStack

import numpy as np

import concourse.bass as bass
import concourse.mybir as mybir
from concourse.bass_utils import run_bass_kernel_spmd

# Problem constants (hardcoded per harness contract)
B, C, L = 8, 8, 4096
F, K = 32, 10
PROC, STEP = 20, 5
NW = 815          # windows actually computed == chan_outlen
SEG = PROC + 1    # 1 separator + 20 cells
NWC = 408         # windows per chunk; 2 chunks = 816 >= 815
NCHUNK = 2
TFREE = NWC * SEG # 2856 scan length
BIG = 30000.0     # fp16-safe sentinel (max DP value ~70)
SLOTS = 2
UNITS = [(cc, wc) for cc in range(2) for wc in range(NCHUNK)]

F32 = mybir.dt.float32
F16 = mybir.dt.float16


def _build_nc(reps: int = 1) -> bass.Bass:
    # detect_race_conditions=False: CoreSim's detector does not model
    # same-engine program order, which this kernel relies on throughout.
    nc = bass.Bass("TRN2", debug=False, detect_race_conditions=False)
    x_d = nc.dram_tensor("x", [C, L], F32, kind="ExternalInput").ap()
    k_d = nc.dram_tensor("negk", [F, K], F32, kind="ExternalInput").ap()
    out_d = nc.dram_tensor("out", [C * F, NWC * NCHUNK], F32,
                           kind="ExternalOutput").ap()

    UNITS_R = UNITS * reps
    big = BIG

    # --- semaphore bookkeeping (python-side op counts) ---
    # DVE emission order: init memsets, then per unit pair, per row i:
    # m(A,i), m(B,i), scan(A,i), scan(B,i)  (m omitted for i=0).
    dve_ops = []  # ("m"|"scan", u, i)
    nu = len(UNITS) * reps
    for base in range(0, nu, 2):
        pair = [base] + ([base + 1] if base + 1 < nu else [])
        for i in range(K):
            if i > 0:
                for u in pair:
                    dve_ops.append(("m", u, i))
            for u in pair:
                dve_ops.append(("scan", u, i))
    N_INIT_MS = 8 + 2 * SLOTS  # memsets before the op stream
    _scan_pos = {(u, i): N_INIT_MS + n + 1
                 for n, (kind, u, i) in enumerate(dve_ops)
                 if kind == "scan"}

    def dve_through_scan(u, i):
        return _scan_pos[(u, i)]

    # ACT order: pair-interleaved to match the DVE order: per pair,
    # squares (u0,i),(u1,i) for each i, then both extract copies.
    act_ops = []  # ("sq"|"cp", u, i)
    for base in range(0, nu, 2):
        pair = [base] + ([base + 1] if base + 1 < nu else [])
        for i in range(K):
            for u in pair:
                act_ops.append(("sq", u, i))
        for u in pair:
            act_ops.append(("cp", u, 0))
    _sq_pos = {(u, i): n + 1 for n, (kind, u, i) in enumerate(act_ops)
               if kind == "sq"}
    _cp_pos = {u: n + 1 for n, (kind, u, i) in enumerate(act_ops)
               if kind == "cp"}

    def act_through_square(u, i):
        return _sq_pos[(u, i)]

    def act_through_copy(u):
        return _cp_pos[u]

    def dma_through_out(u):  # X1 init DMA then one out-DMA per unit
        return 16 * (2 + u)

    with ExitStack() as ctx:
        sb = lambda shape, name, dt: ctx.enter_context(
            nc.sbuf_tensor(name, shape, dt))
        X = [sb([128, L], f"Xt{cc}", F32) for cc in range(2)]
        negK = sb([128, K], "negKt", F32)
        m0 = sb([128, TFREE], "m0t", F16)
        S = [[sb([128, TFREE], f"St{s}_{i}", F16) for i in range(2)]
             for s in range(SLOTS)]
        D = [[sb([128, TFREE], f"Dt{s}_{i}", F16) for i in range(2)]
             for s in range(SLOTS)]
        OB = [sb([128, NWC], f"OBt{s}", F32) for s in range(SLOTS)]

        dma_sem = ctx.enter_context(nc.semaphore("dma_sem"))
        dma0_sem = ctx.enter_context(nc.semaphore("dma0_sem"))
        act_sem = ctx.enter_context(nc.semaphore("act_sem"))
        dve_sem = ctx.enter_context(nc.semaphore("dve_sem"))
        block = ctx.enter_context(nc.Block())

        @block.sync
        def _(sync):
            # negK + X0 first so cc0 compute starts before X1 lands.
            # X[cc] partition p holds x[4*cc + p//32, :] (source AP
            # replicates each channel row 32x via a step-0 dim)
            ksrc = bass.AP(k_d.tensor, 0, [[0, 4], [K, F], [1, K]])
            sync.dma_start(negK.ap(), ksrc).then_inc(dma0_sem, 16)
            for cc in range(2):
                src = bass.AP(x_d.tensor, 4 * cc * L,
                              [[L, 4], [0, 32], [1, L]])
                sync.dma_start(X[cc].ap(), src).then_inc(
                    dma0_sem if cc == 0 else dma_sem, 16)
            for u, (cc, wc) in enumerate(UNITS_R):
                s = u % SLOTS
                sync.wait_ge(act_sem, act_through_copy(u))
                sync.dma_start(
                    out_d[128 * cc:128 * (cc + 1),
                          NWC * wc:NWC * (wc + 1)],
                    OB[s].ap()).then_inc(dma_sem, 16)

        @block.vector
        def _(vector):
            # init: m0 = BIG with 0 at each segment's cell j=0 (offset 1);
            # D separator columns BIG; S separator columns BIG (the
            # in-place scan reads them as data0 on the first unit).
            vector.memset(m0.ap(), big).then_inc(dve_sem, 1)
            m0_seg = m0.ap().rearrange("p (w s) -> p w s", s=SEG)
            vector.memset(m0_seg[:, :, 1], 0.0).then_inc(dve_sem, 1)
            for s in range(SLOTS):
                for i in range(2):
                    d_seg = D[s][i].ap().rearrange("p (w s) -> p w s", s=SEG)
                    vector.memset(d_seg[:, :, 0], big).then_inc(dve_sem, 1)
                    s_seg = S[s][i].ap().rearrange("p (w s) -> p w s", s=SEG)
                    vector.memset(s_seg[:, :, 0], big).then_inc(dve_sem, 1)
            # pad memset count to N_INIT_MS
            for _ in range(N_INIT_MS - 2 - 4 * SLOTS):
                vector.memset(m0_seg[:, :1, 1], 0.0).then_inc(dve_sem, 1)
            act_waited = 0
            for kind, u, i in dve_ops:
                s = u % SLOTS
                if kind == "m":
                    # m into the scan's own output buffer (cells only);
                    # row i-2's values there are dead.  Guard the one
                    # buffer ACT extract-reads (S[s][1]) against the
                    # previous unit's pending cp.
                    if i == 1 and u >= SLOTS:
                        need = act_through_copy(u - SLOTS)
                        if need > act_waited:
                            vector.wait_ge(act_sem, need)
                            act_waited = need
                    prev = S[s][(i - 1) % 2].ap()
                    dst = S[s][i % 2].ap()
                    vector.tensor_tensor(
                        dst[:, 1:], prev[:, 1:], prev[:, :-1],
                        mybir.AluOpType.min).then_inc(dve_sem, 1)
                    continue
                # scan row i: data0 = m0 (i=0) or in-place S[s][i%2]
                m_ap = m0.ap() if i == 0 else S[s][i % 2].ap()
                need = act_through_square(u, i)
                if need > act_waited:
                    vector.wait_ge(act_sem, need)
                    act_waited = need
                vector.tensor_tensor_scan(
                    S[s][i % 2].ap(), m_ap, D[s][i % 2].ap(),
                    float(big),
                    op0=mybir.AluOpType.min,
                    op1=mybir.AluOpType.add).then_inc(dve_sem, 1)

        @block.scalar
        def _(scalar):
            scalar.wait_ge(dma0_sem, 32)  # negK + X0
            dve_waited = 0
            dma_waited = 0
            x1_waited = False
            for kind, u, i in act_ops:
                cc, wc = UNITS_R[u]
                s = u % SLOTS
                if cc == 1 and not x1_waited:
                    scalar.wait_ge(dma_sem, 16)  # X1
                    x1_waited = True
                if kind == "sq":
                    xt = X[cc].ap()
                    win = bass.AP(xt.tensor, xt.offset + 5 * NWC * wc,
                                  [list(xt.ap[0]), [5, NWC], [1, PROC]])
                    # WAR: D[s][i%2] was last read by an earlier scan
                    if i >= 2:
                        need = dve_through_scan(u, i - 2)
                    elif u >= SLOTS:
                        need = dve_through_scan(u - SLOTS, 8 + i)
                    else:
                        need = 0
                    if need > dve_waited:
                        scalar.wait_ge(dve_sem, need)
                        dve_waited = need
                    d_seg = D[s][i % 2].ap().rearrange(
                        "p (w s) -> p w s", s=SEG)
                    scalar.activation(
                        d_seg[:, :, 1:], win,
                        mybir.ActivationFunctionType.Square,
                        bias=negK.ap()[:, i:i + 1],
                        scale=1.0).then_inc(act_sem, 1)
                else:
                    # extract: cell j=19 lives at segment offset 20; final
                    # row (i=9, odd) lands in S[s][1]
                    need = dve_through_scan(u, K - 1)
                    if need > dve_waited:
                        scalar.wait_ge(dve_sem, need)
                        dve_waited = need
                    if u >= SLOTS:
                        dneed = dma_through_out(u - SLOTS)
                        if dneed > dma_waited:
                            scalar.wait_ge(dma_sem, dneed)
                            dma_waited = dneed
                    s_seg = S[s][1].ap().rearrange("p (w s) -> p w s",
                                                   s=SEG)
                    scalar.copy(OB[s].ap(), s_seg[:, :, SEG - 1]).then_inc(
                        act_sem, 1)
    return nc


_NC_CACHE = None


def kernel(x: np.ndarray, kernels: np.ndarray) -> np.ndarray:
    global _NC_CACHE
    if _NC_CACHE is None:
        _NC_CACHE = _build_nc()
    nc = _NC_CACHE
    x = np.ascontiguousarray(x, dtype=np.float32)
    negk = np.ascontiguousarray(-np.asarray(kernels, dtype=np.float32))
    in_maps = [{"x": x[b], "negk": negk} for b in range(B)]
    res = run_bass_kernel_spmd(nc, in_maps, core_ids=list(range(B)))
    out = np.stack([res.results[b]["out"] for b in range(B)], axis=0)
    return out[:, :, :NW]



# revision 2
# speedup vs baseline: 1.7320x; 1.7320x over previous
"""DTW layer (short kernel) Trainium2 Bass kernel — wavefront version.

Problem: x (B=8, C=8, L=4096) f32, kernels (F=32, K=10) f32.
For each (b, c, f, w): DTW cost between kernels[f] (len 10) and window
x[b, c, 5w : 5w+20], for w in [0, 815). Output (B, C*F, 815) f32.

Sharding: data-parallel over batch - core b computes batch b entirely
(C*F = 256 (c,f) combos = 2 partition chunks cc of 128).

Algorithm (per core): anti-diagonal wavefront over the K x PROC DP:
    acc[i,j] = D[i,j] + min(acc[i,j-1], acc[i-1,j], acc[i-1,j-1])
Cells of diagonal d = i+j are independent; with windows stacked
contiguously in the free dim ([slot, w] layout, w innermost), every
operand of the three tensor_tensor ops per diagonal
    t  = min(left, up)        (adjacent slots of A_{d-1})
    t2 = min(t, diag)         (slot of A_{d-2})
    A_d = t2 + D_view         (strided-outer view into D rows)
is contiguous in its innermost dim, so the DVE runs them 2x-packed in
fp16 (~0.4 ns/elem) instead of tensor_tensor_scan's ~2 ns/elem.

DVE per-op cost has a ~600ns floor, so instruction count matters as
much as elements: WT=272 windows per tile (6 tiles/core) maximizes op
size under the SBUF budget; A buffers carry a persistent BIG sentinel
in slot 0 (data cell idx lives in slot idx+1), folding the i=0 border
into the main ops; dve_sem is bumped once per diagonal (not per op).

D rows live in an 11-entry ring of [20*WT] row buffers ([j][w] layout,
w contiguous). ACT fills row (cc,t,i) = Square(xT + (-k_i)) reading
xT, a once-per-cc fp16 copy of x in [j][w-global] layout (so fill
reads are contiguous); DVE consumes row i across diagonals i..i+19.

Raw bass: engines programmed directly with wait_ge + per-engine
semaphores (act_sem, dve_sem, dma0_sem, dma_sem).
"""

from contextlib import ExitStack

import numpy as np

import concourse.bass as bass
import concourse.mybir as mybir
from concourse.bass_utils import run_bass_kernel_spmd

# Problem constants (hardcoded per harness contract)
B, C, L = 8, 8, 4096
F, K = 32, 10
PROC, STEP = 20, 5
NW = 815            # real windows; slot 815 computed and discarded
WSLOT = 816
WT = 272            # windows per tile
NT = WSLOT // WT    # tiles per cc
NSLOT = 12          # D-row ring slots
ROWSZ = PROC * WT   # elements per D row buffer
NDIAG = K + PROC - 1  # 29
TILE_SEM = NDIAG + 1  # dve_sem bumps per tile (per-diag + OB copy)
BIG = 30000.0

F32 = mybir.dt.float32
F16 = mybir.dt.float16
MIN = mybir.AluOpType.min
ADD = mybir.AluOpType.add


def _runs(gbase, ia, ib):
    """Split row range [ia, ib] (ring slots (gbase+i) % NSLOT) into
    ring-contiguous runs."""
    runs = []
    start = ia
    while start <= ib:
        s = (gbase + start) % NSLOT
        span = min(NSLOT - s, ib - start + 1)
        runs.append((start, start + span - 1))
        start += span
    return runs


def _gbase(rep, cc, t):
    return (((rep * 2 + cc) * NT) + t) * K


def _tile_base(rep, cc, t):
    return (((rep * 2 + cc) * NT) + t) * TILE_SEM


def _build_nc(reps: int = 1) -> bass.Bass:
    nc = bass.Bass("TRN2", debug=False, detect_race_conditions=False)
    x_d = nc.dram_tensor("x", [C, L], F32, kind="ExternalInput").ap()
    k_d = nc.dram_tensor("negk", [F, K], F32, kind="ExternalInput").ap()
    out_d = nc.dram_tensor("out", [C * F, WSLOT], F32,
                           kind="ExternalOutput").ap()

    # ---- schedule bookkeeping ----
    # ACT stream: per (rep, cc): [xT build] + NT*K fills, +1 act_sem each.
    def act_pos_fill(rep, cc, t, i):
        return ((rep * 2 + cc) * NT + t) * (1 + K) + 1 + i + 1

    def act_pos_xt(rep, cc, t):
        return ((rep * 2 + cc) * NT + t) * (1 + K) + 1

    # DVE stream: dve_sem += 1 after each diagonal's last op and after
    # each tile's OB copy -> TILE_SEM bumps per tile.
    def dve_pos_diag(rep, cc, t, d):
        return _tile_base(rep, cc, t) + d + 1

    def dve_pos_out(rep, cc, t):
        return _tile_base(rep, cc, t) + TILE_SEM

    with ExitStack() as ctx:
        sb = lambda shape, name, dt: ctx.enter_context(
            nc.sbuf_tensor(name, shape, dt))
        X = sb([128, L], "Xt", F32)
        negK = sb([128, K], "negKt", F32)
        xT = sb([128, PROC * WT], "xTt", F16)
        Dr = sb([128, NSLOT * ROWSZ], "Drt", F16)
        A = [sb([128, (K + 1) * WT], f"At{q}", F16) for q in range(3)]
        Tb = sb([128, K * WT], "Tbt", F16)
        OB = sb([128, WSLOT], "OBt", F32)

        dma0_sem = ctx.enter_context(nc.semaphore("dma0_sem"))
        dma_sem = ctx.enter_context(nc.semaphore("dma_sem"))
        act_sem = ctx.enter_context(nc.semaphore("act_sem"))
        dve_sem = ctx.enter_context(nc.semaphore("dve_sem"))
        block = ctx.enter_context(nc.Block())

        def dcell(rep, cc, t, i, j):
            """2D AP [p, WT] of D element (row i, col j)."""
            s = (_gbase(rep, cc, t) + i) % NSLOT
            base = s * ROWSZ + j * WT
            return Dr.ap()[:, base:base + WT]

        def dview(rep, cc, t, d, ia, ib):
            """D elements (i, d-i), i in [ia, ib]: [(3D AP, cnt), ...]
            split at ring wrap."""
            out = []
            gb = _gbase(rep, cc, t)
            for i0, i1 in _runs(gb, ia, ib):
                s0 = (gb + i0) % NSLOT
                base = s0 * ROWSZ + (d - i0) * WT
                cnt = i1 - i0 + 1
                ap = Dr.ap()
                out.append((bass.AP(
                    ap.tensor, ap.offset + base,
                    [list(ap.ap[0]), [ROWSZ - WT, cnt], [1, WT]]), cnt))
            return out

        def slot(buf, a, b):
            """2D slice of A buf covering slots [a, b]."""
            return buf[:, a * WT:(b + 1) * WT]

        @block.sync
        def _(sync):
            ksrc = bass.AP(k_d.tensor, 0, [[0, 4], [K, F], [1, K]])
            sync.dma_start(negK.ap(), ksrc).then_inc(dma0_sem, 16)
            src0 = bass.AP(x_d.tensor, 0, [[L, 4], [0, 32], [1, L]])
            sync.dma_start(X.ap(), src0).then_inc(dma0_sem, 16)
            # X reload for cc1 once xT0 is built
            sync.wait_ge(act_sem, act_pos_xt(0, 0, NT - 1))
            src1 = bass.AP(x_d.tensor, 4 * L, [[L, 4], [0, 32], [1, L]])
            sync.dma_start(X.ap(), src1).then_inc(dma_sem, 16)
            for rep in range(reps):
                for cc in range(2):
                    sync.wait_ge(dve_sem,
                                 dve_pos_out(rep, cc, NT - 1))
                    sync.dma_start(
                        out_d[128 * cc:128 * (cc + 1), :],
                        OB.ap()).then_inc(dma_sem, 16)

        @block.scalar
        def _(scalar):
            dve_waited = 0
            scalar.wait_ge(dma0_sem, 32)   # negK + X0
            for rep in range(reps):
                for cc in range(2):
                    if rep == 0 and cc == 1:
                        scalar.wait_ge(dma_sem, 16)   # X1 loaded
                    for t in range(NT):
                        # xT build for this tile: [j][w] layout, fp16
                        xsrc = bass.AP(
                            X.ap().tensor,
                            X.ap().offset + 5 * t * WT,
                            [list(X.ap().ap[0]), [1, PROC], [5, WT]])
                        xdst = xT.ap().rearrange("p (j w) -> p j w",
                                                 w=WT)
                        scalar.copy(xdst, xsrc).then_inc(act_sem, 1)
                        for i in range(K):
                            g = _gbase(rep, cc, t) + i
                            gp = g - NSLOT
                            if gp >= 0:
                                rpp, rr = divmod(gp, 2 * NT * K)
                                ccp, rr = divmod(rr, NT * K)
                                tp, ip = divmod(rr, K)
                                need = dve_pos_diag(
                                    rpp, ccp, tp, ip + (PROC - 1))
                                if need > dve_waited:
                                    scalar.wait_ge(dve_sem, need)
                                    dve_waited = need
                            src = bass.AP(
                                xT.ap().tensor, xT.ap().offset,
                                [list(xT.ap().ap[0]), [WT, PROC],
                                 [1, WT]])
                            s = g % NSLOT
                            dst = Dr.ap()[:, s * ROWSZ:(s + 1) * ROWSZ]
                            scalar.activation(
                                dst.rearrange("p (j w) -> p j w", w=WT),
                                src,
                                mybir.ActivationFunctionType.Square,
                                bias=negK.ap()[:, i:i + 1],
                                scale=1.0).then_inc(act_sem, 1)

        @block.vector
        def _(vector):
            act_waited = 0
            dma_waited = 0

            def wait_act(need):
                nonlocal act_waited
                if need > act_waited:
                    vector.wait_ge(act_sem, need)
                    act_waited = need

            # persistent BIG sentinels in slot 0 of each A buf
            for q in range(3):
                vector.memset(A[q].ap()[:, 0:WT], BIG)

            def op3(Ad, rep, cc, t, d, ia, ib, last_inc):
                """A_d[slots 1..] = Tb + D(i=ia..ib, j=d-i); bumps
                dve_sem on the final op when last_inc."""
                views = dview(rep, cc, t, d, ia, ib)
                off = 0
                for q, (dv, cnt) in enumerate(views):
                    inst = vector.tensor_tensor(
                        slot(Ad, 1 + off, off + cnt).rearrange(
                            "p (a b) -> p a b", b=WT),
                        Tb.ap()[:, off * WT:(off + cnt) * WT].rearrange(
                            "p (a b) -> p a b", b=WT),
                        dv, ADD)
                    if last_inc and q == len(views) - 1:
                        inst.then_inc(dve_sem, 1)
                    off += cnt

            for rep in range(reps):
                for cc in range(2):
                    # OB shared: previous (rep,cc) stream's out-DMA done
                    k = 2 * rep + cc
                    if k > 0:
                        need = 16 * (1 + k)
                        if need > dma_waited:
                            vector.wait_ge(dma_sem, need)
                            dma_waited = need
                    for t in range(NT):
                        for d in range(NDIAG):
                            lo, hi = max(0, d - (PROC - 1)), min(K - 1, d)
                            nd = hi - lo + 1
                            Ad = A[d % 3].ap()
                            Ap = A[(d - 1) % 3].ap()
                            App = A[(d - 2) % 3].ap()
                            wait_act(act_pos_fill(rep, cc, t, hi))
                            if d == 0:
                                vector.tensor_copy(
                                    slot(Ad, 1, 1),
                                    dcell(rep, cc, t, 0, 0)
                                ).then_inc(dve_sem, 1)
                                continue
                            if d == 1:
                                vector.tensor_tensor(
                                    slot(Ad, 1, 1), slot(Ap, 1, 1),
                                    dcell(rep, cc, t, 0, 1), ADD)
                                vector.tensor_tensor(
                                    slot(Ad, 2, 2), slot(Ap, 1, 1),
                                    dcell(rep, cc, t, 1, 0),
                                    ADD).then_inc(dve_sem, 1)
                                continue
                            if d <= K - 1:
                                # main cells idx 0..d-1 (i=idx),
                                # out slots 1..d; sentinel covers idx 0
                                vector.tensor_tensor(
                                    Tb.ap()[:, 0:d * WT],
                                    slot(Ap, 1, d),
                                    slot(Ap, 0, d - 1), MIN)
                                vector.tensor_tensor(
                                    Tb.ap()[:, 0:d * WT],
                                    Tb.ap()[:, 0:d * WT],
                                    slot(App, 0, d - 1), MIN)
                                op3(Ad, rep, cc, t, d, 0, d - 1, False)
                                # j=0 edge: idx d (i=d), up only
                                vector.tensor_tensor(
                                    slot(Ad, d + 1, d + 1),
                                    slot(Ap, d, d),
                                    dcell(rep, cc, t, d, 0),
                                    ADD).then_inc(dve_sem, 1)
                                continue
                            if d <= PROC - 1:
                                # main idx 0..9 (i=idx), out slots 1..10
                                vector.tensor_tensor(
                                    Tb.ap()[:, 0:K * WT],
                                    slot(Ap, 1, K),
                                    slot(Ap, 0, K - 1), MIN)
                                vector.tensor_tensor(
                                    Tb.ap()[:, 0:K * WT],
                                    Tb.ap()[:, 0:K * WT],
                                    slot(App, 0, K - 1), MIN)
                                op3(Ad, rep, cc, t, d, 0, K - 1, True)
                                continue
                            # d >= 20: idx 0..nd-1 (i=idx+lo)
                            vector.tensor_tensor(
                                Tb.ap()[:, 0:nd * WT],
                                slot(Ap, 2, nd + 1),
                                slot(Ap, 1, nd), MIN)
                            if d == PROC:
                                vector.tensor_tensor(
                                    Tb.ap()[:, 0:nd * WT],
                                    Tb.ap()[:, 0:nd * WT],
                                    slot(App, 1, nd), MIN)
                            else:
                                vector.tensor_tensor(
                                    Tb.ap()[:, 0:nd * WT],
                                    Tb.ap()[:, 0:nd * WT],
                                    slot(App, 2, nd + 1), MIN)
                            op3(Ad, rep, cc, t, d, lo, hi, True)
                        # tile output: A_28 slot 1 -> OB (f32)
                        vector.tensor_copy(
                            OB.ap()[:, t * WT:(t + 1) * WT],
                            slot(A[(NDIAG - 1) % 3].ap(), 1, 1)
                        ).then_inc(dve_sem, 1)
    return nc


_NC_CACHE = None


def kernel(x: np.ndarray, kernels: np.ndarray) -> np.ndarray:
    global _NC_CACHE
    if _NC_CACHE is None:
        _NC_CACHE = _build_nc()
    nc = _NC_CACHE
    x = np.ascontiguousarray(x, dtype=np.float32)
    negk = np.ascontiguousarray(-np.asarray(kernels, dtype=np.float32))
    in_maps = [{"x": x[b], "negk": negk} for b in range(B)]
    res = run_bass_kernel_spmd(nc, in_maps, core_ids=list(range(B)))
    out = np.stack([res.results[b]["out"] for b in range(B)], axis=0)
    return out[:, :, :NW]
